# revision 10
# baseline (speedup 1.0000x reference)
"""Trainium2 Bass kernel for BroadcastObstaclesToLanes (embedding lookup), v3.

out[m, :] = obs_pos[same_obs_mask[m, 0], :]   m in [0, 16777216)

Sharding: M split across 8 NeuronCores; obs_pos table replicated (gather
fully local per core).

Per core (2,097,152 tokens):
  Host bins tokens by within-block offset o = idx & 31 (32 bins, stable
  order) and pads each bin to a fixed capacity with valid dummy indices
  (block 0). For bin o the gather base is byte offset 8*o into the table,
  the int16 gather index is the 256B-block id q = idx >> 5, and the
  per-token element is just 8 bytes (elem_size=2 f32, stride 256B) - the
  wanted row lands at offset 0, so there is no on-device select at all:
  the gather destination [128, cc, 2] is DMA'd straight to DRAM.

  Each chunk's gather is split across all 4 SWDGE queues (queue q -> Q7
  core pair 2q/2q+1), putting 8 Q7 cores on descriptor generation instead
  of the default 2 (descriptor generation is the dominant cost).

  The host then inverse-permutes the returned bin-major stream into the
  original token order (index relabeling only - all table-value movement
  happens on device).
"""

import numpy as np

N_OBS = 1048576
M_LANES = 16777216
NCORES = 8
MS = M_LANES // NCORES  # 2,097,152 tokens per core
P = 128
NBLK = N_OBS // 32  # 32768 blocks of 256B
NQ = 4  # SWDGE queues
NBINS = 32
NB = 4  # pipeline depth (chunks in flight)

# Per-bin stream layout: NBIG big chunks + one small tail chunk.
GBIG = 32768  # tokens per big chunk (4 x 8192 per queue)
GSMALL = 8192  # tokens per tail chunk (4 x 2048 per queue)
NBIGPB = 2  # big chunks per bin
CAP = NBIGPB * GBIG + GSMALL  # 73728 slots per bin (mean fill 65536)
NCH = NBINS * (NBIGPB + 1)  # total chunks per core
WTOT = NBINS * CAP // 16  # idx columns in the wrapped stream

_cached_nc = None
REPEAT = 1  # program-level repetitions of the full chunk loop (timing use)


def _chunk_list():
    """[(bin, size, col_offset, slot_base, out_name, out_index), ...]"""
    chunks = []
    bi = si = 0
    for b in range(NBINS):
        base = b * CAP
        for k in range(NBIGPB):
            chunks.append((b, GBIG, (base + k * GBIG) // 16, base + k * GBIG,
                           "outB", bi))
            bi += 1
        chunks.append((b, GSMALL, (base + NBIGPB * GBIG) // 16,
                       base + NBIGPB * GBIG, "outS", si))
        si += 1
    return chunks


def _dma_gather_raw(gp, out_ap, in_ap, idxs_ap, num_idxs, elem_size,
                    elem_step, queue_num):
    """dma_gather with elem_size_bytes < 256 (non-transpose path).

    Replicates BassGpSimd.dma_gather's instruction construction minus the
    `elem_size_bytes % 256 == 0` assert: the non-transpose ucode path
    (dma_gather.cpp gen_descs) carries arbitrary descriptor byte lengths;
    256 only constrains the xbar transpose path.
    """
    from concourse import mybir

    gp._assert_queue_num(queue_num)
    assert idxs_ap.dtype == mybir.dt.int16
    assert in_ap.dtype == out_ap.dtype
    dt_sz = mybir.dt.size(in_ap.dtype)
    stride_bytes = elem_step * dt_sz
    assert stride_bytes % 256 == 0
    stride_bytes_256 = stride_bytes // 256
    assert 0 < stride_bytes_256 < 256
    assert in_ap.ap[0][0] == elem_step
    _in_ap = gp.lower_ap_dma(in_ap, for_custom_bir_dma=True)
    _idxs_ap = gp.lower_ap(idxs_ap)
    _out_ap = gp.lower_ap(out_ap)
    inst = gp.add_instruction(
        mybir.InstDMAGatherAnt(
            name=gp.bass.get_next_instruction_name(),
            ins=[*_in_ap, _idxs_ap, gp.lower_val_access(gp.to_reg(num_idxs))],
            outs=[_out_ap],
            transpose=False,
            num_idxs=num_idxs,
            elem_size=elem_size,
            stride_bytes_256=stride_bytes_256,
            gen_mode=0,
            single_packet=False,
            queue_num=queue_num,
            sbuf_tokens_per_rank=0,
            sbuf_free_dim_per_rank=0,
            sbuf_free_dim_pad_per_rank=0,
            sbuf_byte_offset=0,
        )
    )
    return inst


def _dma_gather_raw_sbuf(gp, out_ap, in_ap, idxs_ap, num_idxs, byte_off,
                         queue_num):
    """SBUF-source dma_gather, non-transpose, 8-byte elements.

    The bass API restricts SBUF-source gathers to transpose=True, but the
    ucode's src_is_sbuf branch is transpose-independent (hardware-verified
    exact). Table layout: 256B block q at partition q & 127, rank q >> 7
    (sbuf_free_dim_per_rank=256); sbuf_byte_offset selects the row within
    the block.
    """
    from concourse import mybir

    gp._assert_queue_num(queue_num)
    assert idxs_ap.dtype == mybir.dt.int16
    inst = gp.add_instruction(
        mybir.InstDMAGatherAnt(
            name=gp.bass.get_next_instruction_name(),
            ins=[gp.lower_ap(in_ap), gp.lower_ap(idxs_ap),
                 gp.lower_val_access(gp.to_reg(num_idxs))],
            outs=[gp.lower_ap(out_ap)],
            transpose=False,
            num_idxs=num_idxs,
            elem_size=2,
            stride_bytes_256=0,
            gen_mode=0,
            single_packet=False,
            queue_num=queue_num,
            sbuf_tokens_per_rank=128,
            sbuf_free_dim_per_rank=256,
            sbuf_free_dim_pad_per_rank=0,
            sbuf_byte_offset=byte_off,
        )
    )
    return inst


def _build():
    global _cached_nc
    if _cached_nc is not None:
        return _cached_nc

    import concourse.bacc as bacc
    import concourse.bass as bass
    from concourse import mybir
    from concourse.library_config import mlp
    from contextlib import ExitStack

    nc = bacc.Bacc(
        "TRN2",
        target_bir_lowering=False,
        debug=False,
        num_devices=NCORES,
        num_swdge_queues=NQ,
        dynamic_dma_scratch_size=32768,
    )
    tbl = nc.dram_tensor(
        "tblS", [P, NBLK // 2], mybir.dt.float32, kind="ExternalInput"
    )
    q16_d = nc.dram_tensor(
        "q16", [P, WTOT], mybir.dt.int16, kind="ExternalInput"
    )
    outB = nc.dram_tensor(
        "outB", [NBINS * NBIGPB, P, GBIG // P, 2], mybir.dt.float32,
        kind="ExternalOutput",
    )
    outS = nc.dram_tensor(
        "outS", [NBINS, P, GSMALL // P, 2], mybir.dt.float32,
        kind="ExternalOutput",
    )

    chunks = _chunk_list() * REPEAT
    CCB = GBIG // P  # dst columns, big chunk (256)
    WB = GBIG // 16  # idx columns, big chunk (2048)

    with ExitStack() as _st:
        block = _st.enter_context(nc.Block())
        f32 = mybir.dt.float32

        tblS = _st.enter_context(nc.sbuf_tensor("tblS_sb", [P, NBLK // 2], f32))
        s_tbl = _st.enter_context(nc.semaphore("s_tbl"))
        dsts = [
            _st.enter_context(nc.sbuf_tensor(f"dst{b}", [P, CCB, 2], f32))
            for b in range(NB)
        ]
        idxs = [
            _st.enter_context(
                nc.sbuf_tensor(f"idx{b}", [P, WB], mybir.dt.int16)
            )
            for b in range(NB)
        ]
        s_idx = [_st.enter_context(nc.semaphore(f"s_idx{b}")) for b in range(NB)]
        s_gat = [_st.enter_context(nc.semaphore(f"s_gat{b}")) for b in range(NB)]
        s_out = [_st.enter_context(nc.semaphore(f"s_out{b}")) for b in range(NB)]

        @block.sync
        def _(sy: bass.BassEngine):
            sy.dma_start(tblS[:], tbl.ap()[:]).then_inc(s_tbl, 16)
            for k, (b, sz, col, slot, oname, oi) in enumerate(chunks):
                w = sz // 16
                if k >= NB:
                    sy.wait_ge(s_gat[k % NB], 16 * NQ * (k // NB))
                sy.dma_start(
                    idxs[k % NB][:, :w], q16_d.ap()[:, col : col + w]
                ).then_inc(s_idx[k % NB], 16)

        @block.gpsimd
        def _(gp: bass.BassGpSimd):
            gp.load_library(mlp)
            gp.wait_ge(s_tbl, 16)
            for k, (b, sz, col, slot, oname, oi) in enumerate(chunks):
                niq = sz // NQ
                cq = niq // P
                wq = niq // 16
                gp.wait_ge(s_idx[k % NB], 16 * (k // NB + 1))
                if k >= NB:
                    gp.wait_ge(s_out[k % NB], 16 * (k // NB))
                for q in range(NQ):
                    _dma_gather_raw_sbuf(
                        gp,
                        dsts[k % NB][:, q * cq : (q + 1) * cq, :],
                        tblS[:],
                        idxs[k % NB][:, q * wq : (q + 1) * wq],
                        niq,
                        byte_off=8 * b,
                        queue_num=q,
                    ).then_inc(s_gat[k % NB], 16)

        @block.scalar
        def _(sc: bass.BassEngine):
            for k, (b, sz, col, slot, oname, oi) in enumerate(chunks):
                cc = sz // P
                sc.wait_ge(s_gat[k % NB], 16 * NQ * (k // NB + 1))
                dst_ap = (
                    dsts[k % NB][:]
                    if sz == GBIG
                    else dsts[k % NB][:, :cc, :]
                )
                tgt = outB.ap()[oi] if oname == "outB" else outS.ap()[oi]
                sc.dma_start(tgt, dst_ap).then_inc(s_out[k % NB], 16)

    nc.compile()
    _cached_nc = nc
    return nc


def _prepare(obs_pos, same_obs_mask):
    """Returns (in_maps, per-core host unscramble info)."""
    tblblk = np.asarray(obs_pos, dtype=np.float32).reshape(NBLK, 64)
    # SBUF layout: 256B block q at partition q & 127, rank q >> 7
    tblS = np.ascontiguousarray(
        tblblk.reshape(NBLK // P, P, 64).transpose(1, 0, 2).reshape(P, NBLK // 2)
    )
    idx32 = np.asarray(same_obs_mask).reshape(-1).astype(np.int32)
    in_maps = []
    unscramble = []
    for c in range(NCORES):
        lanes = idx32[c * MS : (c + 1) * MS]
        o = lanes & 31
        q16 = (lanes >> 5).astype(np.int16)
        order = np.argsort(o, kind="stable")
        counts = np.bincount(o, minlength=NBINS)
        assert counts.max() <= CAP, counts.max()
        starts = np.zeros(NBINS, np.int64)
        starts[1:] = np.cumsum(counts)[:-1]
        # Pad every unused slot with a valid dummy index (block 0): gathers
        # whose index stream ends in -1 sentinels get trimmed by the ucode,
        # and trimmed gathers wedge the SWDGE rings at volume (the ring-space
        # reservation is computed from the pre-trim count). All-valid streams
        # are hardware-validated at full scale; the ~13% dummy-slot overhead
        # is cheap.
        stream = np.zeros((NBINS, CAP), np.int16)
        sorted_q = q16[order]
        for b in range(NBINS):
            cnt = int(counts[b])
            stream[b, :cnt] = sorted_q[starts[b] : starts[b] + cnt]
        flat = stream.reshape(-1)
        q16w = np.tile(np.ascontiguousarray(flat.reshape(-1, 16).T), (8, 1))
        in_maps.append({"tblS": tblS, "q16": q16w})
        # stream position of each sorted token: bin*CAP + rank-in-bin
        pos_sorted = np.repeat(np.arange(NBINS, dtype=np.int64) * CAP, counts) + (
            np.arange(MS, dtype=np.int64) - np.repeat(starts, counts)
        )
        unscramble.append((order, pos_sorted))
    return in_maps, unscramble


def kernel(obs_pos, same_obs_mask):
    from concourse.bass_utils import run_bass_kernel_spmd

    nc = _build()
    in_maps, unscramble = _prepare(obs_pos, same_obs_mask)
    res = run_bass_kernel_spmd(nc, in_maps, core_ids=list(range(NCORES)))
    chunks = _chunk_list()
    outs = []
    for c, r in enumerate(res.results):
        ob = r["outB"]  # [NBINS*NBIGPB, P, GBIG//P, 2]
        os_ = r["outS"]  # [NBINS, P, GSMALL//P, 2]
        stream = np.empty((NBINS * CAP, 2), np.float32)
        for b, sz, col, slot, oname, oi in chunks:
            src = ob[oi] if oname == "outB" else os_[oi]
            # token t of chunk at dst[t%128, t//128] -> transpose to t-major
            stream[slot : slot + sz] = (
                src.transpose(1, 0, 2).reshape(sz, 2)
            )
        order, pos_sorted = unscramble[c]
        out_c = np.empty((MS, 2), np.float32)
        out_c[order] = stream[pos_sorted]
        outs.append(out_c)
    return np.ascontiguousarray(np.concatenate(outs, axis=0))


# revision 11
# speedup vs baseline: 1.3493x; 1.3493x over previous
"""Trainium2 Bass kernel for BroadcastObstaclesToLanes (embedding lookup), v3.

out[m, :] = obs_pos[same_obs_mask[m, 0], :]   m in [0, 16777216)

Sharding: M split across 8 NeuronCores; obs_pos table replicated (gather
fully local per core).

Per core (2,097,152 tokens):
  Host bins tokens by within-block offset o = idx & 31 (32 bins, stable
  order) and pads each bin to a fixed capacity with valid dummy indices
  (block 0). For bin o the gather base is byte offset 8*o into the table,
  the int16 gather index is the 256B-block id q = idx >> 5, and the
  per-token element is just 8 bytes (elem_size=2 f32, stride 256B) - the
  wanted row lands at offset 0, so there is no on-device select at all:
  the gather destination [128, cc, 2] is DMA'd straight to DRAM.

  Each chunk's gather is split across all 4 SWDGE queues (queue q -> Q7
  core pair 2q/2q+1), putting 8 Q7 cores on descriptor generation instead
  of the default 2 (descriptor generation is the dominant cost).

  The host then inverse-permutes the returned bin-major stream into the
  original token order (index relabeling only - all table-value movement
  happens on device).
"""

import numpy as np

N_OBS = 1048576
M_LANES = 16777216
NCORES = 8
MS = M_LANES // NCORES  # 2,097,152 tokens per core
P = 128
NBLK = N_OBS // 32  # 32768 blocks of 256B
NQ = 4  # SWDGE queues
NBINS = 32
NB = 4  # pipeline depth (chunks in flight)

# Per-bin stream layout: NBIG big chunks + one small tail chunk.
GBIG = 16384  # tokens per big chunk (4 x 4096 per queue)
GSMALL = 8192  # tokens per tail chunk (4 x 2048 per queue)
NBIGPB = 4  # big chunks per bin
CAP = NBIGPB * GBIG + GSMALL  # 73728 slots per bin (mean fill 65536)
NCH = NBINS * (NBIGPB + 1)  # total chunks per core
WTOT = NBINS * CAP // 16  # idx columns in the wrapped stream

_cached_nc = None
REPEAT = 1  # program-level repetitions of the full chunk loop (timing use)


def _chunk_list():
    """[(bin, size, col_offset, slot_base, out_name, out_index), ...]"""
    chunks = []
    bi = si = 0
    for b in range(NBINS):
        base = b * CAP
        for k in range(NBIGPB):
            chunks.append((b, GBIG, (base + k * GBIG) // 16, base + k * GBIG,
                           "outB", bi))
            bi += 1
        chunks.append((b, GSMALL, (base + NBIGPB * GBIG) // 16,
                       base + NBIGPB * GBIG, "outS", si))
        si += 1
    return chunks


def _dma_gather_raw(gp, out_ap, in_ap, idxs_ap, num_idxs, elem_size,
                    elem_step, queue_num):
    """dma_gather with elem_size_bytes < 256 (non-transpose path).

    Replicates BassGpSimd.dma_gather's instruction construction minus the
    `elem_size_bytes % 256 == 0` assert: the non-transpose ucode path
    (dma_gather.cpp gen_descs) carries arbitrary descriptor byte lengths;
    256 only constrains the xbar transpose path.
    """
    from concourse import mybir

    gp._assert_queue_num(queue_num)
    assert idxs_ap.dtype == mybir.dt.int16
    assert in_ap.dtype == out_ap.dtype
    dt_sz = mybir.dt.size(in_ap.dtype)
    stride_bytes = elem_step * dt_sz
    assert stride_bytes % 256 == 0
    stride_bytes_256 = stride_bytes // 256
    assert 0 < stride_bytes_256 < 256
    assert in_ap.ap[0][0] == elem_step
    _in_ap = gp.lower_ap_dma(in_ap, for_custom_bir_dma=True)
    _idxs_ap = gp.lower_ap(idxs_ap)
    _out_ap = gp.lower_ap(out_ap)
    inst = gp.add_instruction(
        mybir.InstDMAGatherAnt(
            name=gp.bass.get_next_instruction_name(),
            ins=[*_in_ap, _idxs_ap, gp.lower_val_access(gp.to_reg(num_idxs))],
            outs=[_out_ap],
            transpose=False,
            num_idxs=num_idxs,
            elem_size=elem_size,
            stride_bytes_256=stride_bytes_256,
            gen_mode=0,
            single_packet=False,
            queue_num=queue_num,
            sbuf_tokens_per_rank=0,
            sbuf_free_dim_per_rank=0,
            sbuf_free_dim_pad_per_rank=0,
            sbuf_byte_offset=0,
        )
    )
    return inst


def _dma_gather_raw_sbuf(gp, out_ap, in_ap, idxs_ap, num_idxs, byte_off,
                         queue_num):
    """SBUF-source dma_gather, non-transpose, 8-byte elements.

    The bass API restricts SBUF-source gathers to transpose=True, but the
    ucode's src_is_sbuf branch is transpose-independent (hardware-verified
    exact). Table layout: 256B block q at partition q & 127, rank q >> 7
    (sbuf_free_dim_per_rank=256); sbuf_byte_offset selects the row within
    the block.
    """
    from concourse import mybir

    gp._assert_queue_num(queue_num)
    assert idxs_ap.dtype == mybir.dt.int16
    inst = gp.add_instruction(
        mybir.InstDMAGatherAnt(
            name=gp.bass.get_next_instruction_name(),
            ins=[gp.lower_ap(in_ap), gp.lower_ap(idxs_ap),
                 gp.lower_val_access(gp.to_reg(num_idxs))],
            outs=[gp.lower_ap(out_ap)],
            transpose=False,
            num_idxs=num_idxs,
            elem_size=2,
            stride_bytes_256=0,
            gen_mode=0,
            single_packet=False,
            queue_num=queue_num,
            sbuf_tokens_per_rank=128,
            sbuf_free_dim_per_rank=256,
            sbuf_free_dim_pad_per_rank=0,
            sbuf_byte_offset=byte_off,
        )
    )
    return inst


def _build():
    global _cached_nc
    if _cached_nc is not None:
        return _cached_nc

    import concourse.bacc as bacc
    import concourse.bass as bass
    from concourse import mybir
    from concourse.library_config import mlp
    from contextlib import ExitStack

    nc = bacc.Bacc(
        "TRN2",
        target_bir_lowering=False,
        debug=False,
        num_devices=NCORES,
        num_swdge_queues=NQ,
        # 257-descriptor gathers need 16.45KB of ring; the default 16KB
        # carveout forces a mid-instruction await_space stall per gather.
        dynamic_dma_scratch_size=32768,
    )
    tbl = nc.dram_tensor(
        "tblS", [P, NBLK // 2], mybir.dt.float32, kind="ExternalInput"
    )
    q16_d = nc.dram_tensor(
        "q16", [P, WTOT], mybir.dt.int16, kind="ExternalInput"
    )
    outB = nc.dram_tensor(
        "outB", [NBINS * NBIGPB, P, GBIG // P, 2], mybir.dt.float32,
        kind="ExternalOutput",
    )
    outS = nc.dram_tensor(
        "outS", [NBINS, P, GSMALL // P, 2], mybir.dt.float32,
        kind="ExternalOutput",
    )

    chunks = _chunk_list() * REPEAT
    CCB = GBIG // P  # dst columns, big chunk (256)
    WB = GBIG // 16  # idx columns, big chunk (2048)

    with ExitStack() as _st:
        block = _st.enter_context(nc.Block())
        f32 = mybir.dt.float32

        tblS = _st.enter_context(nc.sbuf_tensor("tblS_sb", [P, NBLK // 2], f32))
        s_tbl = _st.enter_context(nc.semaphore("s_tbl"))
        dsts = [
            _st.enter_context(nc.sbuf_tensor(f"dst{b}", [P, CCB, 2], f32))
            for b in range(NB)
        ]
        idxs = [
            _st.enter_context(
                nc.sbuf_tensor(f"idx{b}", [P, WB], mybir.dt.int16)
            )
            for b in range(NB)
        ]
        s_idx = [_st.enter_context(nc.semaphore(f"s_idx{b}")) for b in range(NB)]
        s_gat = [_st.enter_context(nc.semaphore(f"s_gat{b}")) for b in range(NB)]
        s_out = [_st.enter_context(nc.semaphore(f"s_out{b}")) for b in range(NB)]

        @block.sync
        def _(sy: bass.BassEngine):
            sy.dma_start(tblS[:], tbl.ap()[:]).then_inc(s_tbl, 16)
            for k, (b, sz, col, slot, oname, oi) in enumerate(chunks):
                w = sz // 16
                if k >= NB:
                    sy.wait_ge(s_gat[k % NB], 16 * NQ * (k // NB))
                sy.dma_start(
                    idxs[k % NB][:, :w], q16_d.ap()[:, col : col + w]
                ).then_inc(s_idx[k % NB], 16)

        @block.gpsimd
        def _(gp: bass.BassGpSimd):
            gp.load_library(mlp)
            gp.wait_ge(s_tbl, 16)
            for k, (b, sz, col, slot, oname, oi) in enumerate(chunks):
                niq = sz // NQ
                cq = niq // P
                wq = niq // 16
                gp.wait_ge(s_idx[k % NB], 16 * (k // NB + 1))
                if k >= NB:
                    gp.wait_ge(s_out[k % NB], 16 * (k // NB))
                for q in range(NQ):
                    _dma_gather_raw_sbuf(
                        gp,
                        dsts[k % NB][:, q * cq : (q + 1) * cq, :],
                        tblS[:],
                        idxs[k % NB][:, q * wq : (q + 1) * wq],
                        niq,
                        byte_off=8 * b,
                        queue_num=q,
                    ).then_inc(s_gat[k % NB], 16)

        @block.scalar
        def _(sc: bass.BassEngine):
            for k, (b, sz, col, slot, oname, oi) in enumerate(chunks):
                cc = sz // P
                sc.wait_ge(s_gat[k % NB], 16 * NQ * (k // NB + 1))
                dst_ap = (
                    dsts[k % NB][:]
                    if sz == GBIG
                    else dsts[k % NB][:, :cc, :]
                )
                tgt = outB.ap()[oi] if oname == "outB" else outS.ap()[oi]
                sc.dma_start(tgt, dst_ap).then_inc(s_out[k % NB], 16)

    nc.compile()
    _cached_nc = nc
    return nc


def _prepare(obs_pos, same_obs_mask):
    """Returns (in_maps, per-core host unscramble info)."""
    tblblk = np.asarray(obs_pos, dtype=np.float32).reshape(NBLK, 64)
    # SBUF layout: 256B block q at partition q & 127, rank q >> 7
    tblS = np.ascontiguousarray(
        tblblk.reshape(NBLK // P, P, 64).transpose(1, 0, 2).reshape(P, NBLK // 2)
    )
    idx32 = np.asarray(same_obs_mask).reshape(-1).astype(np.int32)
    in_maps = []
    unscramble = []
    for c in range(NCORES):
        lanes = idx32[c * MS : (c + 1) * MS]
        o = lanes & 31
        q16 = (lanes >> 5).astype(np.int16)
        order = np.argsort(o, kind="stable")
        counts = np.bincount(o, minlength=NBINS)
        assert counts.max() <= CAP, counts.max()
        starts = np.zeros(NBINS, np.int64)
        starts[1:] = np.cumsum(counts)[:-1]
        # Pad every unused slot with a valid dummy index (block 0): gathers
        # whose index stream ends in -1 sentinels get trimmed by the ucode,
        # and trimmed gathers wedge the SWDGE rings at volume (the ring-space
        # reservation is computed from the pre-trim count). All-valid streams
        # are hardware-validated at full scale; the ~13% dummy-slot overhead
        # is cheap.
        stream = np.zeros((NBINS, CAP), np.int16)
        sorted_q = q16[order]
        for b in range(NBINS):
            cnt = int(counts[b])
            stream[b, :cnt] = sorted_q[starts[b] : starts[b] + cnt]
        flat = stream.reshape(-1)
        q16w = np.tile(np.ascontiguousarray(flat.reshape(-1, 16).T), (8, 1))
        in_maps.append({"tblS": tblS, "q16": q16w})
        # stream position of each sorted token: bin*CAP + rank-in-bin
        pos_sorted = np.repeat(np.arange(NBINS, dtype=np.int64) * CAP, counts) + (
            np.arange(MS, dtype=np.int64) - np.repeat(starts, counts)
        )
        unscramble.append((order, pos_sorted))
    return in_maps, unscramble


def kernel(obs_pos, same_obs_mask):
    from concourse.bass_utils import run_bass_kernel_spmd

    nc = _build()
    in_maps, unscramble = _prepare(obs_pos, same_obs_mask)
    res = run_bass_kernel_spmd(nc, in_maps, core_ids=list(range(NCORES)))
    chunks = _chunk_list()
    outs = []
    for c, r in enumerate(res.results):
        ob = r["outB"]  # [NBINS*NBIGPB, P, GBIG//P, 2]
        os_ = r["outS"]  # [NBINS, P, GSMALL//P, 2]
        stream = np.empty((NBINS * CAP, 2), np.float32)
        for b, sz, col, slot, oname, oi in chunks:
            src = ob[oi] if oname == "outB" else os_[oi]
            # token t of chunk at dst[t%128, t//128] -> transpose to t-major
            stream[slot : slot + sz] = (
                src.transpose(1, 0, 2).reshape(sz, 2)
            )
        order, pos_sorted = unscramble[c]
        out_c = np.empty((MS, 2), np.float32)
        out_c[order] = stream[pos_sorted]
        outs.append(out_c)
    return np.ascontiguousarray(np.concatenate(outs, axis=0))
